# revision 36
# baseline (speedup 1.0000x reference)
"""Bass/Trainium2 kernel for nn_CRF_RNN (mean-field CRF iteration).

Math (derived from the reference):
  The constant-initialized linear layers collapse the model to a scalar
  fixed-point iteration.  With
      orig0[t,n]  = 0.01 * sum_f inputs[t,n,f]
      K2[n,c]     = sum_k kernels[n,c,k]
      denom[n]    = 0.08 + 0.02 * sum_c K2[n,c]
  the output is x broadcast over the feature dim, where
      x <- (orig0 + 0.02 * (x @ K2^T)) / denom     (3 iterations, x0 = orig0)

Key identity used below: writing s[n] = 0.02/denom[n], the update is
      x_next = (y + orig0/0.02) * s      with  y = x @ K2^T,
and orig0/0.02 = 50*A*rowsum(inputs) is iteration-independent, so each
iteration's PSUM bank is PRELOADED with 50*A*ob via one identity-matmul
and the epilogue collapses to a single elementwise multiply by the
broadcast s.

Distribution: kernels is sharded row-wise (output-node dim) over 8 cores.
Each core builds K2^T for its 512-row slice in SBUF (DVE k-reduction ->
bf16 + PE transposes), computes its slice of each mean-field step with PE
matmuls (contraction over the full node dim), and an AllGather assembles
the full x vector between steps.  The transposed full `inputs` tensor is
replicated to every core so x0^T is computed locally (no AllGather-0);
a sacrificial 2-byte AllGather issued at t=0 absorbs the collectives
firmware's first-call overhead while the kernel stream is in flight.
"""

import os
import numpy as np

# Problem constants (hardcoded per harness contract).
T, N, F, D = 32, 4096, 8, 8
NCORES = 8
A = 0.01      # feature layer constant init
B = 0.01      # linear layer constant init
RNN_NUM = 3

_CACHE = {}


def build_program(t=T, n=N, f=F, d=D, ncores=NCORES, n_warm=24):
    """Build + compile the SPMD Bass program (same program for all cores)."""
    import concourse.bass as bass
    import concourse.tile as tile
    from concourse import bacc, mybir
    from concourse.masks import make_identity
    from concourse.tile_rust import add_dep_helper
    from contextlib import ExitStack

    s = n // ncores            # rows of kernels owned per core
    assert s % 128 == 0 and n % 512 == 0 and t <= 32
    ni = s // 128              # 128-row n-subtiles per core
    kc_tiles = n // 128        # contraction tiles (c dim)
    cch = n // 512             # 512-wide c chunks
    dt = mybir.dt.float32
    dtm = mybir.dt.bfloat16
    X = mybir.AxisListType.X
    ADD = mybir.AluOpType.add

    nc = bacc.Bacc(
        "TRN2", target_bir_lowering=False, debug=False, num_devices=ncores
    )
    kern = nc.dram_tensor("kern", [s, n, d], dt, kind="ExternalInput")
    inp = nc.dram_tensor("inp", [t, s, f], dt, kind="ExternalInput")
    out = nc.dram_tensor("out", [t, s], dt, kind="ExternalOutput")

    with ExitStack() as ctx:
        tc = ctx.enter_context(tile.TileContext(nc))
        singles = ctx.enter_context(tc.tile_pool(name="singles", bufs=1))
        raws = ctx.enter_context(tc.tile_pool(name="raws", bufs=5))
        k2ps = ctx.enter_context(tc.tile_pool(name="k2ps", bufs=4))
        k2tp = ctx.enter_context(tc.tile_pool(name="k2tp", bufs=1))
        xpool = ctx.enter_context(tc.tile_pool(name="xpool", bufs=2))
        small = ctx.enter_context(tc.tile_pool(name="small", bufs=2))
        tpps = ctx.enter_context(tc.tile_pool(name="tpps", bufs=3, space="PSUM"))
        ypps = ctx.enter_context(tc.tile_pool(name="ypps", bufs=1, space="PSUM"))
        opps = ctx.enter_context(tc.tile_pool(name="opps", bufs=1, space="PSUM"))
        dram = ctx.enter_context(tc.tile_pool(name="dram", bufs=2, space="DRAM"))

        # constants first: the stream-phase PE transposes need ident almost
        # immediately, so nothing may delay it on the gpsimd queue
        ident = singles.tile([128, 128], dt, tag="ident", name="ident")
        make_identity(nc, ident)
        ident_b = singles.tile([128, 128], dtm, tag="ident_b", name="ident_b")
        make_identity(nc, ident_b)
        ones_k = singles.tile([128, 1], dtm, tag="ones_k", name="ones_k")
        nc.vector.memset(ones_k, 1.0)
        ones_m = singles.tile([1, t], dtm, tag="ones_m", name="ones_m")
        nc.vector.memset(ones_m, 1.0)

        # ---- local feature reductions (all small DMAs ride SWDGE/gpsimd
        # so they never block the kernel-streaming HWDGE FIFO) ----
        # own slice, natural layout: fifty_ob = 50*A*rowsum = b/s preload
        ind = singles.tile([t, s, f], dt, tag="ind", name="ind")
        nc.gpsimd.dma_start(out=ind, in_=inp.ap())
        o_raw = singles.tile([t, s], dt, tag="o_raw", name="o_raw")
        nc.vector.tensor_reduce(o_raw, ind, axis=X, op=ADD)
        fifty_ob = singles.tile([t, s], dtm, tag="fifty_ob", name="fifty_ob")
        nc.scalar.mul(fifty_ob, o_raw, float(50.0 * A))

        # ---- sacrificial warmup collective: absorbs the collectives
        # firmware's expensive first call (~40us) under the kernel stream.
        warm_src = singles.tile([1, 1], dtm, tag="warm_src", name="warm_src")
        nc.vector.memset(warm_src, 1.0)
        warm_in = dram.tile([1, 1], dtm, tag="warm_in", name="warm_in")
        nc.gpsimd.dma_start(out=warm_in, in_=warm_src)
        warm_out = dram.tile([ncores, 1], dtm, tag="warm_out", name="warm_out")
        nc.gpsimd.collective_compute(
            "AllGather",
            mybir.AluOpType.bypass,
            replica_groups=[list(range(ncores))],
            ins=[warm_in.opt()],
            outs=[warm_out.opt()],
        )

        # ---- x0^T via AllGather (rides behind the warmup collective, fully
        # hidden by the kernel stream; all small DMAs on SWDGE/gpsimd) ----
        xins0 = small.tile([128, ni, t], dtm, tag="xins", name="xins0")
        for jj in range(ni):
            tp0 = tpps.tile([128, t], dt, tag="tp", name="xtp0")
            nc.tensor.transpose(
                tp0, o_raw[:, jj * 128:(jj + 1) * 128], ident[:t, :t]
            )
            nc.scalar.mul(xins0[:, jj, :], tp0, float(A))
        cc_in0 = dram.tile([128, ni, t], dtm, tag="cc_in", name="cc_in0")
        nc.gpsimd.dma_start(out=cc_in0, in_=xins0)
        cc_out0 = dram.tile(
            [ncores, 128, ni, t], dtm, tag="cc_out", name="cc_out0"
        )
        nc.gpsimd.collective_compute(
            "AllGather",
            mybir.AluOpType.bypass,
            replica_groups=[list(range(ncores))],
            ins=[cc_in0.opt()],
            outs=[cc_out0.opt()],
        )
        x0t = xpool.tile([128, kc_tiles, t], dtm, tag="x0t", name="x0t")
        x0v = x0t.rearrange("cl (r j) t -> cl r j t", r=ncores)
        c0v = cc_out0.rearrange("r cl j t -> cl r j t")
        h = ncores // 2
        nc.gpsimd.dma_start(out=x0v[:, :h], in_=c0v[:, :h])
        nc.gpsimd.dma_start(out=x0v[:, h:], in_=c0v[:, h:])

        # ---- PSUM preload for iteration 0: y_ps0 = 50*A*ob ----
        y_banks = [
            ypps.tile([t, s], dt, tag=f"y{b}", name=f"y_ps{b}")
            for b in range(2)
        ]
        nc.tensor.matmul(
            y_banks[0], lhsT=ident_b[:t, :t], rhs=fifty_ob,
            start=True, stop=False,
        )

        # ---- heavy phase: stream kernels, reduce k (bf16), transpose into
        # K2T; iteration-0 matmuls + l22 rowsums interleaved per chunk ----
        k2t_all = k2tp.tile(
            [128, kc_tiles, s], dtm, tag="k2t_all", name="k2t_all")
        k2t = [k2t_all[:, kc, :] for kc in range(kc_tiles)]
        l22_ps = opps.tile([1, s], dt, tag="l22", name="l22_ps")
        gate_inst = None
        for j in range(cch):
            for i in range(ni):
                k2p = k2ps.tile([128, 512], dt, tag="k2p", name="k2p")
                # alternate the stream across both HWDGE rings (SP + ACT)
                # to hide inter-DMA completion bubbles
                deng = nc.sync if (j * ni + i) % 2 == 0 else nc.scalar
                if j == cch - 1 and i == ni - 1:
                    # split the final tile 4-ways so the stream drains with
                    # fine-grained pipelining (the DVE runs ~one reduce
                    # behind the DMA; small pieces cut that lag)
                    for jj in range(4):
                        rawp = raws.tile(
                            [128, 128, d], dt, tag="rawp", name="rawp")
                        (nc.sync if jj % 2 == 0 else nc.scalar).dma_start(
                            out=rawp,
                            in_=kern.ap()[
                                i * 128:(i + 1) * 128,
                                j * 512 + jj * 128:j * 512 + (jj + 1) * 128,
                                :],
                        )
                        nc.vector.tensor_reduce(
                            k2p[:, jj * 128:(jj + 1) * 128], rawp,
                            axis=X, op=ADD)
                else:
                    raw = raws.tile([128, 512, d], dt, tag="raw", name="raw")
                    deng.dma_start(
                        out=raw,
                        in_=kern.ap()[
                            i * 128:(i + 1) * 128, j * 512:(j + 1) * 512, :],
                    )
                    nc.vector.tensor_reduce(k2p, raw, axis=X, op=ADD)
                tpb = tpps.tile([128, 4, 128], dt, tag="tp", name="tpb")
                for jj in range(4):
                    nc.tensor.transpose(
                        tpb[:, jj, :], k2p[:, jj * 128:(jj + 1) * 128], ident
                    )
                cp = nc.scalar.copy(
                    k2t_all[:, j * 4:(j + 1) * 4, i * 128:(i + 1) * 128], tpb)
                if j == cch // 2 and i == ni - 1:
                    gate_inst = cp
            # l22 rowsum accumulation for this chunk's K2T tiles
            for jj in range(4):
                kc = j * 4 + jj
                nc.tensor.matmul(
                    l22_ps, lhsT=ones_k, rhs=k2t[kc],
                    start=(kc == 0), stop=(kc == kc_tiles - 1),
                )

        # ---- denom -> broadcast s = 0.02/denom over t partitions ----
        # (emitted before the iter-0 matmuls so the tiny broadcast matmul
        #  slots into the PE stream as soon as l22 closes)
        denom_row = singles.tile([1, s], dt, tag="denom_row", name="denom_row")
        nc.scalar.activation(
            denom_row, l22_ps, mybir.ActivationFunctionType.Copy,
            bias=float(A * f), scale=float(2.0 * B),
        )
        recip_row = singles.tile([1, s], dt, tag="recip_row", name="recip_row")
        nc.vector.reciprocal_approx_fast(recip_row, denom_row)
        recip_b = singles.tile([1, s], dtm, tag="recip_b", name="recip_b")
        nc.scalar.copy(recip_b, recip_row)
        bc_ps = opps.tile([t, s], dt, tag="bc", name="bc_ps")
        nc.tensor.matmul(bc_ps, lhsT=ones_m, rhs=recip_b, start=True,
                         stop=True)
        s_bc = singles.tile([t, s], dt, tag="s_bc", name="s_bc")
        nc.scalar.mul(s_bc, bc_ps, float(2.0 * B))      # 0.02 / denom

        # ---- iteration-0 matmuls: deferred past mid-stream so the PE
        # stream never head-of-line blocks on the AG0 result x0t ----
        for kc in range(kc_tiles):
            mm = nc.tensor.matmul(
                y_banks[0], lhsT=x0t[:, kc, :], rhs=k2t[kc],
                start=False, stop=(kc == kc_tiles - 1),
            )
            if kc == 0 and gate_inst is not None:
                add_dep_helper(mm.ins, gate_inst.ins, sync=True,
                               reason="defer iter-0 matmuls past mid-stream")

        # ---- cross-core pre-sync: a 2-byte AllGather triggered near each
        # core's phase-1 end (second-to-last chunk) aligns all cores before
        # AG1 without sitting on the critical path.
        sync_in = dram.tile([1, 1], dtm, tag="sync_in", name="sync_in")
        nc.gpsimd.dma_start(
            out=sync_in, in_=k2t_all[0:1, (cch - 1) * 4 - 1, 384:385])
        sync_out = dram.tile([ncores, 1], dtm, tag="sync_out", name="sync_out")
        nc.gpsimd.collective_compute(
            "AllGather",
            mybir.AluOpType.bypass,
            replica_groups=[list(range(ncores))],
            ins=[sync_in.opt()],
            outs=[sync_out.opt()],
        )

        warm_ps = opps.tile([t, s], dt, tag="warm", name="warm_ps")

        # ---- mean-field iterations ----
        for it in range(RNN_NUM):
            y_ps = y_banks[it % 2]
            last = it == RNN_NUM - 1
            # epilogue: x = (y + 50*A*ob) * s   (PSUM was preloaded)
            x_tn = small.tile([t, s], dt, tag="x_tn", name="x_tn")
            nc.vector.tensor_mul(x_tn, y_ps, s_bc)
            if last:
                nc.sync.dma_start(out=out.ap(), in_=x_tn)
                break

            # pack x^T slices (cl-major) and bounce to DRAM for the AG
            xins = small.tile([128, ni, t], dtm, tag="xins", name="xins")
            for jj in range(ni):
                tp = tpps.tile([128, t], dt, tag="tp", name="xtp")
                nc.tensor.transpose(
                    tp, x_tn[:, jj * 128:(jj + 1) * 128], ident[:t, :t]
                )
                nc.scalar.copy(xins[:, jj, :], tp)
            cc_in = dram.tile([128, ni, t], dtm, tag="cc_in", name="cc_in")
            cc_dma = nc.sync.dma_start(out=cc_in, in_=xins)

            # preload the NEXT iteration's PSUM bank (independent of the AG)
            nc.tensor.matmul(
                y_banks[(it + 1) % 2], lhsT=ident_b[:t, :t], rhs=fifty_ob,
                start=True, stop=False,
            )
            # narrow warm matmuls, gated to start once the bounce DMA is
            # done: they keep the PE p-state ramped across the AG+reload
            # window so the next iteration's matmuls run at full clock,
            # while costing little when the AG completes quickly.
            for w in range(n_warm):
                wm = nc.tensor.matmul(
                    warm_ps[:, :128], lhsT=xins[:, 0, :], rhs=k2t[0][:, :128],
                    start=True, stop=True,
                )
                if w == 0:
                    add_dep_helper(wm.ins, cc_dma.ins, sync=True,
                                   reason="gate warmups at AG trigger")

            cc_out = dram.tile(
                [ncores, 128, ni, t], dtm, tag="cc_out", name="cc_out"
            )
            nc.gpsimd.collective_compute(
                "AllGather",
                mybir.AluOpType.bypass,
                replica_groups=[list(range(ncores))],
                ins=[cc_in.opt()],
                outs=[cc_out.opt()],
            )
            # reload gathered x^T as two independent half-tiles on the two
            # parallel HWDGE rings (SP + ACT): matmuls over the first half
            # start as soon as its reload lands, without waiting for the
            # second half.
            hk = kc_tiles // 2
            hr = ncores // 2
            cv = cc_out.rearrange("r cl j t -> cl r j t")
            xh = []
            for b in range(2):
                xcur = xpool.tile([128, hk, t], dtm, tag=f"xcur{b}",
                                  name=f"xcur{b}")
                xcv = xcur.rearrange("cl (r j) t -> cl r j t", r=hr)
                eng = nc.sync if b == 0 else nc.gpsimd
                eng.dma_start(out=xcv, in_=cv[:, b * hr:(b + 1) * hr])
                xh.append(xcur)

            for kc in range(kc_tiles):
                nc.tensor.matmul(
                    y_banks[(it + 1) % 2], lhsT=xh[kc // hk][:, kc % hk, :],
                    rhs=k2t[kc],
                    start=False, stop=(kc == kc_tiles - 1),
                )

    nc.compile()
    return nc


def _get_program(key=(T, N, F, D, NCORES)):
    if key not in _CACHE:
        _CACHE[key] = build_program(*key)
    return _CACHE[key]


def make_in_maps(inputs_arr, kernels_arr, t=T, n=N, f=F, d=D, ncores=NCORES):
    s = n // ncores
    inputs_arr = np.ascontiguousarray(inputs_arr, dtype=np.float32)
    kernels_arr = np.ascontiguousarray(kernels_arr, dtype=np.float32)
    in_maps = []
    for c in range(ncores):
        in_maps.append({
            "kern": kernels_arr[c * s:(c + 1) * s],
            "inp": np.ascontiguousarray(inputs_arr[:, c * s:(c + 1) * s, :]),
        })
    return in_maps


def run_device(inputs_arr, kernels_arr, trace=False, tmpdir=None):
    from concourse.bass_utils import run_bass_kernel_spmd

    nc = _get_program()
    in_maps = make_in_maps(inputs_arr, kernels_arr)
    res = run_bass_kernel_spmd(
        nc, in_maps, core_ids=list(range(NCORES)), trace=trace, tmpdir=tmpdir
    )
    slices = [res.results[c]["out"] for c in range(NCORES)]
    x = np.concatenate(slices, axis=1)          # (T, N)
    out = np.broadcast_to(x[:, :, None], (T, N, F)).copy()
    return out.astype(np.float32), res


def kernel(**inputs):
    inputs_arr = np.asarray(inputs["inputs"], dtype=np.float32)
    kernels_arr = np.asarray(inputs["kernels"], dtype=np.float32)
    out, _ = run_device(inputs_arr, kernels_arr, trace=False)
    return out


# revision 37
# speedup vs baseline: 1.0416x; 1.0416x over previous
"""Bass/Trainium2 kernel for nn_CRF_RNN (mean-field CRF iteration).

Math (derived from the reference):
  The constant-initialized linear layers collapse the model to a scalar
  fixed-point iteration.  With
      orig0[t,n]  = 0.01 * sum_f inputs[t,n,f]
      K2[n,c]     = sum_k kernels[n,c,k]
      denom[n]    = 0.08 + 0.02 * sum_c K2[n,c]
  the output is x broadcast over the feature dim, where
      x <- (orig0 + 0.02 * (x @ K2^T)) / denom     (3 iterations, x0 = orig0)

Key identity used below: writing s[n] = 0.02/denom[n], the update is
      x_next = (y + orig0/0.02) * s      with  y = x @ K2^T,
and orig0/0.02 = 50*A*rowsum(inputs) is iteration-independent, so each
iteration's PSUM bank is PRELOADED with 50*A*ob via one identity-matmul
and the epilogue collapses to a single elementwise multiply by the
broadcast s.

Distribution: kernels is sharded row-wise (output-node dim) over 8 cores.
Each core builds K2^T for its 512-row slice in SBUF (DVE k-reduction ->
bf16 + PE transposes), computes its slice of each mean-field step with PE
matmuls (contraction over the full node dim), and an AllGather assembles
the full x vector between steps.  The transposed full `inputs` tensor is
replicated to every core so x0^T is computed locally (no AllGather-0);
a sacrificial 2-byte AllGather issued at t=0 absorbs the collectives
firmware's first-call overhead while the kernel stream is in flight.
"""

import os
import numpy as np

# Problem constants (hardcoded per harness contract).
T, N, F, D = 32, 4096, 8, 8
NCORES = 8
A = 0.01      # feature layer constant init
B = 0.01      # linear layer constant init
RNN_NUM = 3

_CACHE = {}


def build_program(t=T, n=N, f=F, d=D, ncores=NCORES, n_warm=24):
    """Build + compile the SPMD Bass program (same program for all cores)."""
    import concourse.bass as bass
    import concourse.tile as tile
    from concourse import bacc, mybir
    from concourse.masks import make_identity
    from concourse.tile_rust import add_dep_helper
    from contextlib import ExitStack

    s = n // ncores            # rows of kernels owned per core
    assert s % 128 == 0 and n % 512 == 0 and t <= 32
    ni = s // 128              # 128-row n-subtiles per core
    kc_tiles = n // 128        # contraction tiles (c dim)
    cch = n // 512             # 512-wide c chunks
    dt = mybir.dt.float32
    dtm = mybir.dt.bfloat16
    X = mybir.AxisListType.X
    ADD = mybir.AluOpType.add

    nc = bacc.Bacc(
        "TRN2", target_bir_lowering=False, debug=False, num_devices=ncores
    )
    kern = nc.dram_tensor("kern", [s, n, d], dt, kind="ExternalInput")
    inp = nc.dram_tensor("inp", [t, s, f], dt, kind="ExternalInput")
    out = nc.dram_tensor("out", [t, s], dt, kind="ExternalOutput")

    with ExitStack() as ctx:
        tc = ctx.enter_context(tile.TileContext(nc))
        singles = ctx.enter_context(tc.tile_pool(name="singles", bufs=1))
        raws = ctx.enter_context(tc.tile_pool(name="raws", bufs=5))
        k2ps = ctx.enter_context(tc.tile_pool(name="k2ps", bufs=4))
        k2tp = ctx.enter_context(tc.tile_pool(name="k2tp", bufs=1))
        xpool = ctx.enter_context(tc.tile_pool(name="xpool", bufs=2))
        small = ctx.enter_context(tc.tile_pool(name="small", bufs=2))
        tpps = ctx.enter_context(tc.tile_pool(name="tpps", bufs=3, space="PSUM"))
        ypps = ctx.enter_context(tc.tile_pool(name="ypps", bufs=1, space="PSUM"))
        opps = ctx.enter_context(tc.tile_pool(name="opps", bufs=1, space="PSUM"))
        dram = ctx.enter_context(tc.tile_pool(name="dram", bufs=2, space="DRAM"))

        # constants first: the stream-phase PE transposes need ident almost
        # immediately, so nothing may delay it on the gpsimd queue
        ident = singles.tile([128, 128], dt, tag="ident", name="ident")
        make_identity(nc, ident)
        ident_b = singles.tile([128, 128], dtm, tag="ident_b", name="ident_b")
        make_identity(nc, ident_b)
        ones_k = singles.tile([128, 1], dtm, tag="ones_k", name="ones_k")
        nc.vector.memset(ones_k, 1.0)
        ones_m = singles.tile([1, t], dtm, tag="ones_m", name="ones_m")
        nc.vector.memset(ones_m, 1.0)

        # ---- local feature reductions (all small DMAs ride SWDGE/gpsimd
        # so they never block the kernel-streaming HWDGE FIFO) ----
        # own slice, natural layout: fifty_ob = 50*A*rowsum = b/s preload
        ind = singles.tile([t, s, f], dt, tag="ind", name="ind")
        nc.gpsimd.dma_start(out=ind, in_=inp.ap())
        o_raw = singles.tile([t, s], dt, tag="o_raw", name="o_raw")
        nc.vector.tensor_reduce(o_raw, ind, axis=X, op=ADD)
        fifty_ob = singles.tile([t, s], dtm, tag="fifty_ob", name="fifty_ob")
        nc.scalar.mul(fifty_ob, o_raw, float(50.0 * A))

        # ---- sacrificial warmup collective: absorbs the collectives
        # firmware's expensive first call (~40us) under the kernel stream.
        warm_src = singles.tile([1, 1], dtm, tag="warm_src", name="warm_src")
        nc.vector.memset(warm_src, 1.0)
        warm_in = dram.tile([1, 1], dtm, tag="warm_in", name="warm_in")
        nc.gpsimd.dma_start(out=warm_in, in_=warm_src)
        warm_out = dram.tile([ncores, 1], dtm, tag="warm_out", name="warm_out")
        nc.gpsimd.collective_compute(
            "AllGather",
            mybir.AluOpType.bypass,
            replica_groups=[list(range(ncores))],
            ins=[warm_in.opt()],
            outs=[warm_out.opt()],
        )

        # ---- x0^T via AllGather (rides behind the warmup collective, fully
        # hidden by the kernel stream; all small DMAs on SWDGE/gpsimd) ----
        xins0 = small.tile([128, ni, t], dtm, tag="xins", name="xins0")
        for jj in range(ni):
            tp0 = tpps.tile([128, t], dt, tag="tp", name="xtp0")
            nc.tensor.transpose(
                tp0, o_raw[:, jj * 128:(jj + 1) * 128], ident[:t, :t]
            )
            nc.scalar.mul(xins0[:, jj, :], tp0, float(A))
        cc_in0 = dram.tile([128, ni, t], dtm, tag="cc_in", name="cc_in0")
        nc.gpsimd.dma_start(out=cc_in0, in_=xins0)
        cc_out0 = dram.tile(
            [ncores, 128, ni, t], dtm, tag="cc_out", name="cc_out0"
        )
        nc.gpsimd.collective_compute(
            "AllGather",
            mybir.AluOpType.bypass,
            replica_groups=[list(range(ncores))],
            ins=[cc_in0.opt()],
            outs=[cc_out0.opt()],
        )
        x0t = xpool.tile([128, kc_tiles, t], dtm, tag="x0t", name="x0t")
        x0v = x0t.rearrange("cl (r j) t -> cl r j t", r=ncores)
        c0v = cc_out0.rearrange("r cl j t -> cl r j t")
        h = ncores // 2
        nc.gpsimd.dma_start(out=x0v[:, :h], in_=c0v[:, :h])
        nc.gpsimd.dma_start(out=x0v[:, h:], in_=c0v[:, h:])

        # ---- PSUM preload for iteration 0: y_ps0 = 50*A*ob ----
        y_banks = [
            ypps.tile([t, s], dt, tag=f"y{b}", name=f"y_ps{b}")
            for b in range(2)
        ]
        nc.tensor.matmul(
            y_banks[0], lhsT=ident_b[:t, :t], rhs=fifty_ob,
            start=True, stop=False,
        )

        # ---- heavy phase: stream kernels, reduce k (bf16), transpose into
        # K2T; iteration-0 matmuls + l22 rowsums interleaved per chunk ----
        k2t_all = k2tp.tile(
            [128, kc_tiles, s], dtm, tag="k2t_all", name="k2t_all")
        k2t = [k2t_all[:, kc, :] for kc in range(kc_tiles)]
        l22_ps = opps.tile([1, s], dt, tag="l22", name="l22_ps")
        gate_inst = None
        for j in range(cch):
            for i in range(ni):
                k2p = k2ps.tile([128, 512], dt, tag="k2p", name="k2p")
                if j == cch - 1 and i == ni - 1:
                    # split the final tile 4-ways so the stream drains with
                    # fine-grained pipelining (the DVE runs ~one reduce
                    # behind the DMA; small pieces cut that lag)
                    for jj in range(4):
                        rawp = raws.tile(
                            [128, 128, d], dt, tag="rawp", name="rawp")
                        nc.sync.dma_start(
                            out=rawp,
                            in_=kern.ap()[
                                i * 128:(i + 1) * 128,
                                j * 512 + jj * 128:j * 512 + (jj + 1) * 128,
                                :],
                        )
                        nc.vector.tensor_reduce(
                            k2p[:, jj * 128:(jj + 1) * 128], rawp,
                            axis=X, op=ADD)
                else:
                    raw = raws.tile([128, 512, d], dt, tag="raw", name="raw")
                    nc.sync.dma_start(
                        out=raw,
                        in_=kern.ap()[
                            i * 128:(i + 1) * 128, j * 512:(j + 1) * 512, :],
                    )
                    nc.vector.tensor_reduce(k2p, raw, axis=X, op=ADD)
                tpb = tpps.tile([128, 4, 128], dt, tag="tp", name="tpb")
                for jj in range(4):
                    nc.tensor.transpose(
                        tpb[:, jj, :], k2p[:, jj * 128:(jj + 1) * 128], ident
                    )
                cp = nc.scalar.copy(
                    k2t_all[:, j * 4:(j + 1) * 4, i * 128:(i + 1) * 128], tpb)
                if j == cch // 2 and i == ni - 1:
                    gate_inst = cp
            # l22 rowsum accumulation for this chunk's K2T tiles
            for jj in range(4):
                kc = j * 4 + jj
                nc.tensor.matmul(
                    l22_ps, lhsT=ones_k, rhs=k2t[kc],
                    start=(kc == 0), stop=(kc == kc_tiles - 1),
                )

        # ---- denom -> broadcast s = 0.02/denom over t partitions ----
        # (emitted before the iter-0 matmuls so the tiny broadcast matmul
        #  slots into the PE stream as soon as l22 closes)
        denom_row = singles.tile([1, s], dt, tag="denom_row", name="denom_row")
        nc.scalar.activation(
            denom_row, l22_ps, mybir.ActivationFunctionType.Copy,
            bias=float(A * f), scale=float(2.0 * B),
        )
        recip_row = singles.tile([1, s], dt, tag="recip_row", name="recip_row")
        nc.vector.reciprocal_approx_fast(recip_row, denom_row)
        recip_b = singles.tile([1, s], dtm, tag="recip_b", name="recip_b")
        nc.scalar.copy(recip_b, recip_row)
        bc_ps = opps.tile([t, s], dt, tag="bc", name="bc_ps")
        nc.tensor.matmul(bc_ps, lhsT=ones_m, rhs=recip_b, start=True,
                         stop=True)
        s_bc = singles.tile([t, s], dt, tag="s_bc", name="s_bc")
        nc.scalar.mul(s_bc, bc_ps, float(2.0 * B))      # 0.02 / denom

        # ---- iteration-0 matmuls: deferred past mid-stream so the PE
        # stream never head-of-line blocks on the AG0 result x0t ----
        for kc in range(kc_tiles):
            mm = nc.tensor.matmul(
                y_banks[0], lhsT=x0t[:, kc, :], rhs=k2t[kc],
                start=False, stop=(kc == kc_tiles - 1),
            )
            if kc == 0 and gate_inst is not None:
                add_dep_helper(mm.ins, gate_inst.ins, sync=True,
                               reason="defer iter-0 matmuls past mid-stream")

        # ---- cross-core pre-sync: a 2-byte AllGather triggered near each
        # core's phase-1 end (second-to-last chunk) aligns all cores before
        # AG1 without sitting on the critical path.
        sync_in = dram.tile([1, 1], dtm, tag="sync_in", name="sync_in")
        nc.gpsimd.dma_start(
            out=sync_in, in_=k2t_all[0:1, (cch - 1) * 4 - 1, 384:385])
        sync_out = dram.tile([ncores, 1], dtm, tag="sync_out", name="sync_out")
        nc.gpsimd.collective_compute(
            "AllGather",
            mybir.AluOpType.bypass,
            replica_groups=[list(range(ncores))],
            ins=[sync_in.opt()],
            outs=[sync_out.opt()],
        )

        warm_ps = opps.tile([t, s], dt, tag="warm", name="warm_ps")

        # ---- mean-field iterations ----
        for it in range(RNN_NUM):
            y_ps = y_banks[it % 2]
            last = it == RNN_NUM - 1
            # epilogue: x = (y + 50*A*ob) * s   (PSUM was preloaded)
            x_tn = small.tile([t, s], dt, tag="x_tn", name="x_tn")
            nc.vector.tensor_mul(x_tn, y_ps, s_bc)
            if last:
                nc.sync.dma_start(out=out.ap(), in_=x_tn)
                break

            # pack x^T slices (cl-major) and bounce to DRAM for the AG
            xins = small.tile([128, ni, t], dtm, tag="xins", name="xins")
            for jj in range(ni):
                tp = tpps.tile([128, t], dt, tag="tp", name="xtp")
                nc.tensor.transpose(
                    tp, x_tn[:, jj * 128:(jj + 1) * 128], ident[:t, :t]
                )
                nc.scalar.copy(xins[:, jj, :], tp)
            cc_in = dram.tile([128, ni, t], dtm, tag="cc_in", name="cc_in")
            cc_dma = nc.sync.dma_start(out=cc_in, in_=xins)

            # preload the NEXT iteration's PSUM bank (independent of the AG)
            nc.tensor.matmul(
                y_banks[(it + 1) % 2], lhsT=ident_b[:t, :t], rhs=fifty_ob,
                start=True, stop=False,
            )
            # narrow warm matmuls, gated to start once the bounce DMA is
            # done: they keep the PE p-state ramped across the AG+reload
            # window so the next iteration's matmuls run at full clock,
            # while costing little when the AG completes quickly.
            for w in range(n_warm):
                wm = nc.tensor.matmul(
                    warm_ps[:, :128], lhsT=xins[:, 0, :], rhs=k2t[0][:, :128],
                    start=True, stop=True,
                )
                if w == 0:
                    add_dep_helper(wm.ins, cc_dma.ins, sync=True,
                                   reason="gate warmups at AG trigger")

            cc_out = dram.tile(
                [ncores, 128, ni, t], dtm, tag="cc_out", name="cc_out"
            )
            nc.gpsimd.collective_compute(
                "AllGather",
                mybir.AluOpType.bypass,
                replica_groups=[list(range(ncores))],
                ins=[cc_in.opt()],
                outs=[cc_out.opt()],
            )
            # reload gathered x^T as two independent half-tiles on the two
            # parallel HWDGE rings (SP + ACT): matmuls over the first half
            # start as soon as its reload lands, without waiting for the
            # second half.
            hk = kc_tiles // 2
            hr = ncores // 2
            cv = cc_out.rearrange("r cl j t -> cl r j t")
            xh = []
            for b in range(2):
                xcur = xpool.tile([128, hk, t], dtm, tag=f"xcur{b}",
                                  name=f"xcur{b}")
                xcv = xcur.rearrange("cl (r j) t -> cl r j t", r=hr)
                eng = nc.sync if b == 0 else nc.gpsimd
                eng.dma_start(out=xcv, in_=cv[:, b * hr:(b + 1) * hr])
                xh.append(xcur)

            for kc in range(kc_tiles):
                nc.tensor.matmul(
                    y_banks[(it + 1) % 2], lhsT=xh[kc // hk][:, kc % hk, :],
                    rhs=k2t[kc],
                    start=False, stop=(kc == kc_tiles - 1),
                )

    nc.compile()
    return nc


def _get_program(key=(T, N, F, D, NCORES)):
    if key not in _CACHE:
        _CACHE[key] = build_program(*key)
    return _CACHE[key]


def make_in_maps(inputs_arr, kernels_arr, t=T, n=N, f=F, d=D, ncores=NCORES):
    s = n // ncores
    inputs_arr = np.ascontiguousarray(inputs_arr, dtype=np.float32)
    kernels_arr = np.ascontiguousarray(kernels_arr, dtype=np.float32)
    in_maps = []
    for c in range(ncores):
        in_maps.append({
            "kern": kernels_arr[c * s:(c + 1) * s],
            "inp": np.ascontiguousarray(inputs_arr[:, c * s:(c + 1) * s, :]),
        })
    return in_maps


def run_device(inputs_arr, kernels_arr, trace=False, tmpdir=None):
    from concourse.bass_utils import run_bass_kernel_spmd

    nc = _get_program()
    in_maps = make_in_maps(inputs_arr, kernels_arr)
    res = run_bass_kernel_spmd(
        nc, in_maps, core_ids=list(range(NCORES)), trace=trace, tmpdir=tmpdir
    )
    slices = [res.results[c]["out"] for c in range(NCORES)]
    x = np.concatenate(slices, axis=1)          # (T, N)
    out = np.broadcast_to(x[:, :, None], (T, N, F)).copy()
    return out.astype(np.float32), res


def kernel(**inputs):
    inputs_arr = np.asarray(inputs["inputs"], dtype=np.float32)
    kernels_arr = np.asarray(inputs["kernels"], dtype=np.float32)
    out, _ = run_device(inputs_arr, kernels_arr, trace=False)
    return out


# revision 45
# speedup vs baseline: 1.0654x; 1.0229x over previous
"""Bass/Trainium2 kernel for nn_CRF_RNN (mean-field CRF iteration).

Math (derived from the reference):
  The constant-initialized linear layers collapse the model to a scalar
  fixed-point iteration.  With
      orig0[t,n]  = 0.01 * sum_f inputs[t,n,f]
      K2[n,c]     = sum_k kernels[n,c,k]
      denom[n]    = 0.08 + 0.02 * sum_c K2[n,c]
  the output is x broadcast over the feature dim, where
      x <- (orig0 + 0.02 * (x @ K2^T)) / denom     (3 iterations, x0 = orig0)

Key identity used below: writing s[n] = 0.02/denom[n], the update is
      x_next = (y + orig0/0.02) * s      with  y = x @ K2^T,
and orig0/0.02 = 50*A*rowsum(inputs) is iteration-independent, so each
iteration's PSUM bank is PRELOADED with 50*A*ob via one identity-matmul
and the epilogue collapses to a single elementwise multiply by the
broadcast s.

Distribution: kernels is sharded row-wise (output-node dim) over 8 cores.
Each core builds K2^T for its 512-row slice in SBUF (DVE k-reduction ->
bf16 + PE transposes), computes its slice of each mean-field step with PE
matmuls (contraction over the full node dim), and an AllGather assembles
the full x vector between steps.  The transposed full `inputs` tensor is
replicated to every core so x0^T is computed locally (no AllGather-0);
a sacrificial 2-byte AllGather issued at t=0 absorbs the collectives
firmware's first-call overhead while the kernel stream is in flight.
"""

import os
import numpy as np

# Problem constants (hardcoded per harness contract).
T, N, F, D = 32, 4096, 8, 8
NCORES = 8
A = 0.01      # feature layer constant init
B = 0.01      # linear layer constant init
RNN_NUM = 3

_CACHE = {}


def build_program(t=T, n=N, f=F, d=D, ncores=NCORES, n_warm=24):
    """Build + compile the SPMD Bass program (same program for all cores)."""
    import concourse.bass as bass
    import concourse.tile as tile
    from concourse import bacc, mybir
    from concourse.masks import make_identity
    from concourse.tile_rust import add_dep_helper
    from contextlib import ExitStack

    s = n // ncores            # rows of kernels owned per core
    assert s % 128 == 0 and n % 512 == 0 and t <= 32
    ni = s // 128              # 128-row n-subtiles per core
    kc_tiles = n // 128        # contraction tiles (c dim)
    cch = n // 512             # 512-wide c chunks
    dt = mybir.dt.float32
    dtm = mybir.dt.bfloat16
    X = mybir.AxisListType.X
    ADD = mybir.AluOpType.add

    nc = bacc.Bacc(
        "TRN2", target_bir_lowering=False, debug=False, num_devices=ncores
    )
    kern = nc.dram_tensor("kern", [s, n, d], dt, kind="ExternalInput")
    inp = nc.dram_tensor("inp", [t, s, f], dt, kind="ExternalInput")
    out = nc.dram_tensor("out", [t, s], dt, kind="ExternalOutput")

    with ExitStack() as ctx:
        tc = ctx.enter_context(tile.TileContext(nc))
        singles = ctx.enter_context(tc.tile_pool(name="singles", bufs=1))
        raws = ctx.enter_context(tc.tile_pool(name="raws", bufs=5))
        rawhs = ctx.enter_context(tc.tile_pool(name="rawhs", bufs=2))
        k2ps = ctx.enter_context(tc.tile_pool(name="k2ps", bufs=4))
        k2tp = ctx.enter_context(tc.tile_pool(name="k2tp", bufs=1))
        xpool = ctx.enter_context(tc.tile_pool(name="xpool", bufs=2))
        small = ctx.enter_context(tc.tile_pool(name="small", bufs=2))
        tpps = ctx.enter_context(tc.tile_pool(name="tpps", bufs=3, space="PSUM"))
        ypps = ctx.enter_context(tc.tile_pool(name="ypps", bufs=1, space="PSUM"))
        opps = ctx.enter_context(tc.tile_pool(name="opps", bufs=1, space="PSUM"))
        dram = ctx.enter_context(tc.tile_pool(name="dram", bufs=2, space="DRAM"))

        # constants first: the stream-phase PE transposes need ident almost
        # immediately, so nothing may delay it on the gpsimd queue
        ident = singles.tile([128, 128], dt, tag="ident", name="ident")
        make_identity(nc, ident)
        ident_b = singles.tile([128, 128], dtm, tag="ident_b", name="ident_b")
        make_identity(nc, ident_b)
        ones_k = singles.tile([128, 1], dtm, tag="ones_k", name="ones_k")
        nc.vector.memset(ones_k, 1.0)
        ones_m = singles.tile([1, t], dtm, tag="ones_m", name="ones_m")
        nc.vector.memset(ones_m, 1.0)

        # ---- local feature reductions (all small DMAs ride SWDGE/gpsimd
        # so they never block the kernel-streaming HWDGE FIFO) ----
        # own slice, natural layout: fifty_ob = 50*A*rowsum = b/s preload
        ind = singles.tile([t, s, f], dt, tag="ind", name="ind")
        nc.gpsimd.dma_start(out=ind, in_=inp.ap())
        o_raw = singles.tile([t, s], dt, tag="o_raw", name="o_raw")
        nc.vector.tensor_reduce(o_raw, ind, axis=X, op=ADD)
        fifty_ob = singles.tile([t, s], dtm, tag="fifty_ob", name="fifty_ob")
        nc.scalar.mul(fifty_ob, o_raw, float(50.0 * A))

        # ---- sacrificial warmup collective: absorbs the collectives
        # firmware's expensive first call (~40us) under the kernel stream.
        warm_src = singles.tile([1, 1], dtm, tag="warm_src", name="warm_src")
        nc.vector.memset(warm_src, 1.0)
        warm_in = dram.tile([1, 1], dtm, tag="warm_in", name="warm_in")
        nc.gpsimd.dma_start(out=warm_in, in_=warm_src)
        warm_out = dram.tile([ncores, 1], dtm, tag="warm_out", name="warm_out")
        nc.gpsimd.collective_compute(
            "AllGather",
            mybir.AluOpType.bypass,
            replica_groups=[list(range(ncores))],
            ins=[warm_in.opt()],
            outs=[warm_out.opt()],
        )

        # ---- x0^T via AllGather (rides behind the warmup collective, fully
        # hidden by the kernel stream; all small DMAs on SWDGE/gpsimd) ----
        xins0 = small.tile([128, ni, t], dtm, tag="xins", name="xins0")
        for jj in range(ni):
            tp0 = tpps.tile([128, t], dt, tag="tp", name="xtp0")
            nc.tensor.transpose(
                tp0, o_raw[:, jj * 128:(jj + 1) * 128], ident[:t, :t]
            )
            nc.scalar.mul(xins0[:, jj, :], tp0, float(A))
        cc_in0 = dram.tile([128, ni, t], dtm, tag="cc_in", name="cc_in0")
        nc.gpsimd.dma_start(out=cc_in0, in_=xins0)
        cc_out0 = dram.tile(
            [ncores, 128, ni, t], dtm, tag="cc_out", name="cc_out0"
        )
        nc.gpsimd.collective_compute(
            "AllGather",
            mybir.AluOpType.bypass,
            replica_groups=[list(range(ncores))],
            ins=[cc_in0.opt()],
            outs=[cc_out0.opt()],
        )
        x0t = xpool.tile([128, kc_tiles, t], dtm, tag="x0t", name="x0t")
        x0v = x0t.rearrange("cl (r j) t -> cl r j t", r=ncores)
        c0v = cc_out0.rearrange("r cl j t -> cl r j t")
        h = ncores // 2
        nc.gpsimd.dma_start(out=x0v[:, :h], in_=c0v[:, :h])
        nc.gpsimd.dma_start(out=x0v[:, h:], in_=c0v[:, h:])

        # ---- PSUM preload for iteration 0: y_ps0 = 50*A*ob ----
        y_banks = [
            ypps.tile([t, s], dt, tag=f"y{b}", name=f"y_ps{b}")
            for b in range(2)
        ]
        nc.tensor.matmul(
            y_banks[0], lhsT=ident_b[:t, :t], rhs=fifty_ob,
            start=True, stop=False,
        )

        # ---- heavy phase: stream kernels, reduce k (bf16), transpose into
        # K2T; iteration-0 matmuls + l22 rowsums interleaved per chunk ----
        k2t_all = k2tp.tile(
            [128, kc_tiles, s], dtm, tag="k2t_all", name="k2t_all")
        k2t = [k2t_all[:, kc, :] for kc in range(kc_tiles)]
        l22_ps = opps.tile([1, s], dt, tag="l22", name="l22_ps")
        gate_inst = None
        for j in range(cch):
            for i in range(ni):
                k2p = k2ps.tile([128, 512], dt, tag="k2p", name="k2p")
                if j == cch - 1 and i == ni - 2:
                    # split the second-to-last tile's DMA 2-ways so the DVE
                    # gets an earlier start on the end-of-stream reduces
                    for jj in range(2):
                        rawh = rawhs.tile(
                            [128, 256, d], dt, tag="rawh", name="rawh")
                        nc.sync.dma_start(
                            out=rawh,
                            in_=kern.ap()[
                                i * 128:(i + 1) * 128,
                                j * 512 + jj * 256:j * 512 + (jj + 1) * 256,
                                :],
                        )
                        nc.vector.tensor_reduce(
                            k2p[:, jj * 256:(jj + 1) * 256], rawh,
                            axis=X, op=ADD)
                elif j == cch - 1 and i == ni - 1:
                    # split the final tile 4-ways so the stream drains with
                    # fine-grained pipelining (the DVE runs ~one reduce
                    # behind the DMA; small pieces cut that lag)
                    for jj in range(4):
                        rawp = raws.tile(
                            [128, 128, d], dt, tag="rawp", name="rawp")
                        nc.sync.dma_start(
                            out=rawp,
                            in_=kern.ap()[
                                i * 128:(i + 1) * 128,
                                j * 512 + jj * 128:j * 512 + (jj + 1) * 128,
                                :],
                        )
                        nc.vector.tensor_reduce(
                            k2p[:, jj * 128:(jj + 1) * 128], rawp,
                            axis=X, op=ADD)
                else:
                    raw = raws.tile([128, 512, d], dt, tag="raw", name="raw")
                    nc.sync.dma_start(
                        out=raw,
                        in_=kern.ap()[
                            i * 128:(i + 1) * 128, j * 512:(j + 1) * 512, :],
                    )
                    nc.vector.tensor_reduce(k2p, raw, axis=X, op=ADD)
                tpb = tpps.tile([128, 4, 128], dt, tag="tp", name="tpb")
                for jj in range(4):
                    nc.tensor.transpose(
                        tpb[:, jj, :], k2p[:, jj * 128:(jj + 1) * 128], ident
                    )
                cp = nc.scalar.copy(
                    k2t_all[:, j * 4:(j + 1) * 4, i * 128:(i + 1) * 128], tpb)
                if j == cch // 2 and i == ni - 1:
                    gate_inst = cp
                if j == cch - 1 and i == 0:
                    late_gate = cp
            # l22 rowsum accumulation for this chunk's K2T tiles
            for jj in range(4):
                kc = j * 4 + jj
                nc.tensor.matmul(
                    l22_ps, lhsT=ones_k, rhs=k2t[kc],
                    start=(kc == 0), stop=(kc == kc_tiles - 1),
                )

        # ---- denom -> broadcast s = 0.02/denom over t partitions ----
        # (emitted before the iter-0 matmuls so the tiny broadcast matmul
        #  slots into the PE stream as soon as l22 closes)
        denom_row = singles.tile([1, s], dt, tag="denom_row", name="denom_row")
        nc.scalar.activation(
            denom_row, l22_ps, mybir.ActivationFunctionType.Copy,
            bias=float(A * f), scale=float(2.0 * B),
        )
        recip_row = singles.tile([1, s], dt, tag="recip_row", name="recip_row")
        nc.vector.reciprocal_approx_fast(recip_row, denom_row)
        recip_b = singles.tile([1, s], dtm, tag="recip_b", name="recip_b")
        nc.scalar.copy(recip_b, recip_row)
        bc_ps = opps.tile([t, s], dt, tag="bc", name="bc_ps")
        nc.tensor.matmul(bc_ps, lhsT=ones_m, rhs=recip_b, start=True,
                         stop=True)
        s_bc = singles.tile([t, s], dt, tag="s_bc", name="s_bc")
        nc.scalar.mul(s_bc, bc_ps, float(2.0 * B))      # 0.02 / denom

        # narrow warm-filler matmuls: keep the PE p-state ramped through the
        # end-of-stream drain so the trailing iter-0 matmuls run at full
        # clock (gated to enter the PE stream at the last chunk)
        warm_ps = opps.tile([t, s], dt, tag="warm", name="warm_ps")
        for w in range(16):
            wm = nc.tensor.matmul(
                warm_ps[:, :128], lhsT=xins0[:, 0, :], rhs=k2t_all[:, 0, :128],
                start=True, stop=True,
            )
            if w == 0:
                add_dep_helper(wm.ins, late_gate.ins, sync=True,
                               reason="gate drain warm-filler at late stream")

        # ---- iteration-0 matmuls: deferred past mid-stream so the PE
        # stream never head-of-line blocks on the AG0 result x0t ----
        for kc in range(kc_tiles):
            mm = nc.tensor.matmul(
                y_banks[0], lhsT=x0t[:, kc, :], rhs=k2t[kc],
                start=False, stop=(kc == kc_tiles - 1),
            )
            if kc == 0 and gate_inst is not None:
                add_dep_helper(mm.ins, gate_inst.ins, sync=True,
                               reason="defer iter-0 matmuls past mid-stream")

        # ---- cross-core pre-sync: a 2-byte AllGather triggered near each
        # core's phase-1 end (second-to-last chunk) aligns all cores before
        # AG1 without sitting on the critical path.
        sync_in = dram.tile([1, 1], dtm, tag="sync_in", name="sync_in")
        nc.gpsimd.dma_start(
            out=sync_in, in_=k2t_all[0:1, (cch - 1) * 4 - 1, 384:385])
        sync_out = dram.tile([ncores, 1], dtm, tag="sync_out", name="sync_out")
        nc.gpsimd.collective_compute(
            "AllGather",
            mybir.AluOpType.bypass,
            replica_groups=[list(range(ncores))],
            ins=[sync_in.opt()],
            outs=[sync_out.opt()],
        )

        # ---- mean-field iterations ----
        for it in range(RNN_NUM):
            y_ps = y_banks[it % 2]
            last = it == RNN_NUM - 1
            # epilogue: x = (y + 50*A*ob) * s   (PSUM was preloaded)
            x_tn = small.tile([t, s], dt, tag="x_tn", name="x_tn")
            nc.vector.tensor_mul(x_tn, y_ps, s_bc)
            if last:
                nc.sync.dma_start(out=out.ap(), in_=x_tn)
                break

            # pack x^T slices (cl-major) and bounce to DRAM for the AG
            xins = small.tile([128, ni, t], dtm, tag="xins", name="xins")
            for jj in range(ni):
                tp = tpps.tile([128, t], dt, tag="tp", name="xtp")
                nc.tensor.transpose(
                    tp, x_tn[:, jj * 128:(jj + 1) * 128], ident[:t, :t]
                )
                nc.scalar.copy(xins[:, jj, :], tp)
            cc_in = dram.tile([128, ni, t], dtm, tag="cc_in", name="cc_in")
            # bounce on gpsimd: the AG trigger (also gpsimd) then follows
            # its own queue without a cross-engine semaphore hop
            cc_dma = nc.gpsimd.dma_start(out=cc_in, in_=xins)

            # preload the NEXT iteration's PSUM bank (independent of the AG)
            nc.tensor.matmul(
                y_banks[(it + 1) % 2], lhsT=ident_b[:t, :t], rhs=fifty_ob,
                start=True, stop=False,
            )
            # narrow warm matmuls, gated to start once the bounce DMA is
            # done: they keep the PE p-state ramped across the AG+reload
            # window so the next iteration's matmuls run at full clock,
            # while costing little when the AG completes quickly.
            for w in range(n_warm):
                wm = nc.tensor.matmul(
                    warm_ps[:, :128], lhsT=xins[:, 0, :], rhs=k2t[0][:, :128],
                    start=True, stop=True,
                )
                if w == 0:
                    add_dep_helper(wm.ins, cc_dma.ins, sync=True,
                                   reason="gate warmups at AG trigger")

            cc_out = dram.tile(
                [ncores, 128, ni, t], dtm, tag="cc_out", name="cc_out"
            )
            nc.gpsimd.collective_compute(
                "AllGather",
                mybir.AluOpType.bypass,
                replica_groups=[list(range(ncores))],
                ins=[cc_in.opt()],
                outs=[cc_out.opt()],
            )
            # reload gathered x^T as two independent half-tiles on the two
            # parallel HWDGE rings (SP + ACT): matmuls over the first half
            # start as soon as its reload lands, without waiting for the
            # second half.
            hk = kc_tiles // 2
            hr = ncores // 2
            cv = cc_out.rearrange("r cl j t -> cl r j t")
            xh = []
            for b in range(2):
                xcur = xpool.tile([128, hk, t], dtm, tag=f"xcur{b}",
                                  name=f"xcur{b}")
                xcv = xcur.rearrange("cl (r j) t -> cl r j t", r=hr)
                eng = nc.sync if b == 0 else nc.gpsimd
                eng.dma_start(out=xcv, in_=cv[:, b * hr:(b + 1) * hr])
                xh.append(xcur)

            for kc in range(kc_tiles):
                nc.tensor.matmul(
                    y_banks[(it + 1) % 2], lhsT=xh[kc // hk][:, kc % hk, :],
                    rhs=k2t[kc],
                    start=False, stop=(kc == kc_tiles - 1),
                )

    nc.compile()
    return nc


def _get_program(key=(T, N, F, D, NCORES)):
    if key not in _CACHE:
        _CACHE[key] = build_program(*key)
    return _CACHE[key]


def make_in_maps(inputs_arr, kernels_arr, t=T, n=N, f=F, d=D, ncores=NCORES):
    s = n // ncores
    inputs_arr = np.ascontiguousarray(inputs_arr, dtype=np.float32)
    kernels_arr = np.ascontiguousarray(kernels_arr, dtype=np.float32)
    in_maps = []
    for c in range(ncores):
        in_maps.append({
            "kern": kernels_arr[c * s:(c + 1) * s],
            "inp": np.ascontiguousarray(inputs_arr[:, c * s:(c + 1) * s, :]),
        })
    return in_maps


def run_device(inputs_arr, kernels_arr, trace=False, tmpdir=None):
    from concourse.bass_utils import run_bass_kernel_spmd

    nc = _get_program()
    in_maps = make_in_maps(inputs_arr, kernels_arr)
    res = run_bass_kernel_spmd(
        nc, in_maps, core_ids=list(range(NCORES)), trace=trace, tmpdir=tmpdir
    )
    slices = [res.results[c]["out"] for c in range(NCORES)]
    x = np.concatenate(slices, axis=1)          # (T, N)
    out = np.broadcast_to(x[:, :, None], (T, N, F)).copy()
    return out.astype(np.float32), res


def kernel(**inputs):
    inputs_arr = np.asarray(inputs["inputs"], dtype=np.float32)
    kernels_arr = np.asarray(inputs["kernels"], dtype=np.float32)
    out, _ = run_device(inputs_arr, kernels_arr, trace=False)
    return out
